# revision 19
# baseline (speedup 1.0000x reference)
"""FBGCN layer kernel for 8 Trainium2 NeuronCores.

out = aL * GCNConv(x, edge_index; W_conv, b_conv) + aH * (Lsym @ relu(x @ W_high.T))

Sharding: 1D row-partition of output nodes across 8 cores (1536 rows each).
The per-core cost is DMA-bandwidth bound (one serial 360 GB/s pipe in the
cost model), so the design minimizes total DMA bytes:

  - High-pass: stream this core's column slice of (256*aH*Lsym).T in
    fp8-e3m4 (18.9 MB, the irreducible floor) as the STATIONARY operand
    against moving fp16 Y = relu(x @ Wh.T) blocks. Output accumulates in
    12 per-block PSUM accumulators sharing banks (memset-seeded,
    start=False). Descale 1/256 on the PSUM->SBUF copy.
  - Low-pass GCN restructured as (S @ x) @ Wc.T with HOST-PACKED
    messages: the host already knows the full slot layout (it builds the
    seg matrix), so it packs msg[slot] = x[src(slot)] in fp8 as a dense
    [128, QBT*64] DRAM tensor. That replaces the old 36us
    256B-descriptor dma_gather (+ scratch round trip) with a ~5us
    full-bandwidth contiguous load. Per 128-target block: seg-matmul
    (fp8 x fp8) accumulates aggT[d,t] = sum_slots msg[s,d]*seg[s,t] in a
    [64,128] PSUM tile, then one [64]x[64,64] matmul applies Wc.T/16 and
    the host-built self-loop+bias term (sbias) is added.
  - Edges sorted by target, one slot per distinct (32-target group,
    source) pair; seg holds 16*aL*dinv[src]*dinv[tgt] in fp8 (G=32
    halves seg bytes vs G=64 at ~7% more slots). Pad slots are zero.
  - lsym loads are issued from the Activation engine's HWDGE queue;
    consts + xT (chunked so A0 starts early) from SP's queue.
No cross-core communication.
"""

import numpy as np

import concourse.bacc as bacc
import concourse.mybir as mybir
import concourse.tile as tile
from concourse.bass_utils import run_bass_kernel_spmd

N, E, D = 12288, 196608, 64
NCORES = 8
M = N // NCORES          # 1536 output rows per core
MB = M // 128            # 12 target blocks per core
KB = N // 128            # 96 contraction blocks
G = 32                   # target-group width
GPB = 128 // G           # groups per block
NG = M // G              # groups per core
SL = 256.0               # lsym fp8 scale (folded with aH on host)
SSEG = 16.0              # seg fp8 scale (descaled into wcs on host)
A0B = 8                  # kb blocks per A0 batch

F32 = mybir.dt.float32
F16 = mybir.dt.float16
E3 = mybir.dt.float8e3
AFT = mybir.ActivationFunctionType


def _build_program(chunk_counts, gcn_sched=None, do_a1=True, do_gcn=True,
                   ls_bufs=10, ls_pack=2, xt_chunks=6, msg_pieces=3,
                   seg_pieces=2, ls_eng="act", const_eng=None, out_eng=None,
                   fin_pieces=((0, 4), (4, 8), (8, 12)), taper=4,
                   msg_f16=False):
    """chunk_counts: tuple of NG per-group chunk counts (same on all cores)."""
    C = list(chunk_counts)
    assert len(C) == NG and all(c >= 1 for c in C)
    coff = np.zeros(NG + 1, np.int64)
    coff[1:] = np.cumsum(C)
    QBT = int(coff[NG])              # total edge chunks per core
    MSGDT = F16 if msg_f16 else E3

    nc = bacc.Bacc("TRN2", target_bir_lowering=False, debug=False,
                   num_devices=NCORES)

    lsymT = nc.dram_tensor("lsymT", [N, M], E3, kind="ExternalInput")
    xT = nc.dram_tensor("xT", [D, N], F16, kind="ExternalInput")
    wt2 = nc.dram_tensor("wt2", [D, 2 * D], F16, kind="ExternalInput")
    msgT = nc.dram_tensor("msgT", [128, QBT * D], MSGDT,
                          kind="ExternalInput")
    segT = nc.dram_tensor("segT", [128, QBT * G], E3, kind="ExternalInput")
    sbiasT = nc.dram_tensor("sbiasT", [128, MB * D], F16,
                            kind="ExternalInput")
    # partition-major output (one contiguous 1536B line per partition -
    # full DMA bandwidth); host reassembles to [M, D]
    outp = nc.dram_tensor("out", [128, MB * D], F16, kind="ExternalOutput")

    ls_dma = {None: nc.sync, "act": nc.scalar}[ls_eng]
    const_dma = {None: nc.sync, "act": nc.scalar}[const_eng]
    out_dma = {None: nc.sync, "act": nc.scalar}[out_eng]

    # block boundaries (chunk index space), for piece-wise msg/seg loads
    bl_off = [int(coff[GPB * b]) for b in range(MB + 1)]

    def piece_bounds(npieces):
        """Split the MB blocks into npieces contiguous runs of blocks."""
        per = -(-MB // npieces)
        return [(bl_off[min(i * per, MB)], bl_off[min((i + 1) * per, MB)])
                for i in range(npieces)]

    with tile.TileContext(nc) as tc:
        with (
            tc.tile_pool(name="consts", bufs=1) as consts,
            tc.tile_pool(name="ls", bufs=ls_bufs) as ls_pool,
            tc.tile_pool(name="aggh", bufs=2) as agg_pool,
            tc.tile_pool(name="psb", bufs=1, space="PSUM") as ps_big,
            tc.tile_pool(name="psa", bufs=2, space="PSUM") as ps_a0,
            tc.tile_pool(name="psg", bufs=2, space="PSUM") as ps_agg,
        ):
            # ---- consts + xT (A0 is the critical-path prologue) ----
            wt2_sb = consts.tile([D, 2 * D], F16, tag="wt2")
            const_dma.dma_start(wt2_sb[:], wt2[:])
            # sbias early: it seeds the hh PSUM accumulator, which must be
            # ready before the first lsym-stream matmul
            sbias = consts.tile([128, MB * D], F16, tag="sbias")
            const_dma.dma_start(sbias[:], sbiasT[:])
            xt_sb = consts.tile([D, N], F16, tag="xt")
            assert KB % xt_chunks == 0
            XTC = N // xt_chunks
            for h in range(xt_chunks):
                nc.sync.dma_start(xt_sb[:, h * XTC:(h + 1) * XTC],
                                  xT[:, h * XTC:(h + 1) * XTC])
            msg_sb = consts.tile([128, QBT * D], MSGDT, tag="msg")
            for c0, c1 in piece_bounds(msg_pieces):
                const_dma.dma_start(msg_sb[:, c0 * D:c1 * D],
                                    msgT[:, c0 * D:c1 * D])
            seg_sb = consts.tile([128, QBT * G], E3, tag="seg")
            for c0, c1 in piece_bounds(seg_pieces):
                const_dma.dma_start(seg_sb[:, c0 * G:c1 * G],
                                    segT[:, c0 * G:c1 * G])
            y16 = consts.tile([128, KB * D], F16, tag="y16")
            ob_sb = consts.tile([128, MB * D], F16, tag="ob")

            # ---- phase A0: Y16 = relu(x@Wh.T)/SL fp16 ----
            # (the 1/SL lsym descale is folded in here so the finale can
            # add hhps + hl directly with no ACT descale hop)
            for bt in range(KB // A0B):
                ps = ps_a0.tile([128, A0B * D], F32, tag="psa")
                for i in range(A0B):
                    kb = bt * A0B + i
                    nc.tensor.matmul(
                        ps[:, i * D:(i + 1) * D],
                        lhsT=xt_sb[:, kb * 128:(kb + 1) * 128],
                        rhs=wt2_sb[:, 0:D],
                        start=True, stop=True,
                    )
                nc.scalar.activation(
                    y16[:, bt * A0B * D:(bt + 1) * A0B * D], ps[:], AFT.Relu,
                    scale=1.0 / SL)

            # ---- high-pass stream + GCN compute interleaved ----
            if gcn_sched is None:
                gcn_sched = [10 + 7 * i for i in range(MB)]
            assert len(gcn_sched) == MB
            sched = {}
            for b, k in enumerate(gcn_sched):
                sched.setdefault(k, []).append(b)

            # 12 per-block accumulators [128, 64] packed in 2 PSUM banks.
            # start=True would mark a whole 2KB zero region pending and
            # corrupt sibling accumulators, so seed + accumulate with
            # start=False (group check skipped). The seed is the host-built
            # self-loop + bias term sbias, and the GCN second-stage matmul
            # also accumulates here, so the finale is a plain fp16 copy.
            hhps = ps_big.tile([128, MB * D], F32, tag="hh")
            nc.vector.tensor_copy(hhps[:], sbias[:])
            hhv = hhps[:].rearrange("p (b f) -> p b f", b=MB)
            segv = seg_sb[:].rearrange("p (q t) -> p q t", t=G)
            msgv = msg_sb[:].rearrange("p (q f) -> p q f", f=D)

            def emit_gcn_block(b):
                # aggT[d, t] accumulator: all GPB groups share one PSUM
                # bank, so memset-seed + start=False (same trick as hhps).
                agg_ps = ps_agg.tile([64, 128], F32, tag="agg")
                nc.vector.memset(agg_ps[:], 0)
                for g in range(GPB):
                    gl = GPB * b + g
                    cg = C[gl]
                    for c in range(cg):
                        q = int(coff[gl]) + c
                        nc.tensor.matmul(
                            agg_ps[:, G * g:G * (g + 1)],
                            lhsT=msgv[:, q, :],
                            rhs=segv[:, q, :],
                            start=False, stop=(c == cg - 1),
                            skip_group_check=True)
                aggh = agg_pool.tile([64, 128], F16, tag="aggh")
                nc.scalar.activation(aggh[:], agg_ps[:], AFT.Copy)
                nc.tensor.matmul(hhv[:, b, :], lhsT=aggh[:],
                                 rhs=wt2_sb[:, D:2 * D],
                                 start=False, stop=False,
                                 skip_group_check=True)

            # lsym load schedule: packs of ls_pack, tapering to singles at
            # the end so the last PE chunk starts sooner after its load
            pack_of = {}
            k = 0
            while k < KB:
                size = ls_pack if k < KB - taper else 1
                pack_of[k] = size
                k += size
            ls_sb = None
            cur0 = 0
            for kb in range(KB if do_a1 else 0):
                if kb in pack_of:
                    size = pack_of[kb]
                    cur0 = kb
                    ls_sb = ls_pool.tile([128, ls_pack * M], E3, tag="ls")
                    if kb == KB - 1:
                        # split the very last load at the finale piece
                        # boundaries so each finale piece starts as soon as
                        # its column range lands
                        for h0, h1 in fin_pieces:
                            ls_dma.dma_start(
                                ls_sb[:, h0 * 128:h1 * 128],
                                lsymT[kb * 128:(kb + 1) * 128,
                                      h0 * 128:h1 * 128]
                                .rearrange("(t p) m -> p (t m)", p=128),
                            )
                    else:
                        ls_dma.dma_start(
                            ls_sb[:, 0:size * M]
                            .rearrange("p (t m) -> p t m", t=size),
                            lsymT[kb * 128:(kb + size) * 128, :]
                            .rearrange("(t p) m -> p t m", p=128),
                        )
                lsv = ls_sb[:].rearrange("p (t m) -> p t m", t=ls_pack)
                for b in range(MB):
                    nc.tensor.matmul(
                        hhv[:, b, :],
                        lhsT=lsv[:, kb - cur0, b * 128:(b + 1) * 128],
                        rhs=y16[:, kb * D:(kb + 1) * D],
                        start=False, stop=(kb == KB - 1),
                        skip_group_check=True,
                    )
                if do_gcn and kb in sched:
                    for b in sched[kb]:
                        emit_gcn_block(b)
            if do_gcn and not do_a1:
                for b in range(MB):
                    emit_gcn_block(b)

            # ---- final: fp16 copy + store (split to overlap) ----
            for h0, h1 in fin_pieces:
                sl = slice(h0 * D, h1 * D)
                nc.scalar.activation(ob_sb[:, sl], hhps[:, sl], AFT.Copy)
                out_dma.dma_start(outp[:, sl], ob_sb[:, sl])

    nc.compile()
    return nc


def _prepare_host(x, edge_index, Lsym, W_high, W_conv, b_conv, aL, aH,
                  msg_f16=False):
    """Shard + preprocess inputs. Returns (in_maps, chunk_counts)."""
    import ml_dtypes
    E3NP = ml_dtypes.float8_e3m4
    MSGNP = np.float16 if msg_f16 else E3NP

    x = np.asarray(x, np.float32)
    edge_index = np.asarray(edge_index)
    Lsym = np.asarray(Lsym, np.float32)
    W_high = np.asarray(W_high, np.float32)
    W_conv = np.asarray(W_conv, np.float32)
    b_conv = np.asarray(b_conv, np.float32)
    aL = float(np.asarray(aL))
    aH = float(np.asarray(aH))

    src = edge_index[0].astype(np.int64)
    tgt = edge_index[1].astype(np.int64)

    # degrees with self loops (matches PyG GCNConv gcn_norm)
    deg = np.bincount(tgt, minlength=N).astype(np.float64) + 1.0
    dinv = 1.0 / np.sqrt(deg)
    w = (aL * dinv[src] * dinv[tgt]).astype(np.float32)
    wself = (aL * dinv * dinv).astype(np.float32)

    grp = tgt // G                    # global group id
    # dedupe (group, src): one msg slot serves every edge from the same
    # source into the group (weights land in different seg columns / sum)
    key = grp * N + src
    uk, inv = np.unique(key, return_inverse=True)
    ugrp = uk // N
    usrc = uk % N
    ucnt = np.bincount(ugrp, minlength=NCORES * NG).reshape(NCORES, NG)
    C = np.maximum(1, -(-ucnt.max(axis=0) // 128)).astype(np.int64)
    coff = np.zeros(NG + 1, np.int64)
    coff[1:] = np.cumsum(C)
    QBT = int(coff[NG])

    ustart = np.zeros(NCORES * NG, np.int64)
    ucnt_flat = np.bincount(ugrp, minlength=NCORES * NG)
    ustart[1:] = np.cumsum(ucnt_flat)[:-1]
    upos = np.arange(len(uk)) - ustart[ugrp]
    ucore = ugrp // NG
    ugl = ugrp % NG

    # host-packed messages: msg[slot] = x[src(slot)], pad slots zero
    x8 = x.astype(MSGNP)
    msg_all = np.zeros((NCORES, 128, QBT, D), MSGNP)
    chunk_u = coff[ugl] + upos // 128
    msg_all[ucore, upos % 128, chunk_u] = x8[usrc]
    msg_all = msg_all.reshape(NCORES, 128, QBT * D)

    # segment matrix, partition-major layout [128, QBT*G], value 16*w
    seg32 = np.zeros((NCORES, 128, QBT * G), np.float32)
    core_e = grp // NG
    pos_e = upos[inv]
    chunk_e = coff[grp % NG] + pos_e // 128
    np.add.at(seg32, (core_e, pos_e % 128, chunk_e * G + tgt % G),
              SSEG * w)
    segT_all = seg32.astype(E3NP)

    xT = np.ascontiguousarray(x.T).astype(np.float16)
    # wt2 = [W_high.T | W_conv.T/SSEG]; the GCN second-stage matmul applies
    # Wc.T with the 1/16 seg descale folded in
    wt2 = np.ascontiguousarray(np.concatenate(
        [W_high.T, W_conv.T / SSEG], axis=1)).astype(np.float16)
    # self-loop + bias term, host-side in fp32 (device cannot index its own
    # core's xw slice under SPMD): sbias[node] = wself*xw[node] + aL*b_conv
    xw_full = x @ W_conv.T.astype(np.float32)
    sb_full = wself[:, None] * xw_full + (aL * b_conv)[None, :]
    # [N, D] -> per core [128, MB*D] with node b*128+p at [p, b*D:(b+1)*D]
    sbias_all = sb_full.reshape(NCORES, MB, 128, D).transpose(0, 2, 1, 3) \
        .reshape(NCORES, 128, MB * D)

    in_maps = []
    for j in range(NCORES):
        lsymT_j = np.ascontiguousarray(
            (SL * aH * Lsym[j * M:(j + 1) * M, :]).T).astype(E3NP)
        in_maps.append({
            "lsymT": lsymT_j,
            "xT": xT,
            "wt2": wt2,
            "msgT": np.ascontiguousarray(msg_all[j]),
            "segT": np.ascontiguousarray(segT_all[j]),
            "sbiasT": np.ascontiguousarray(sbias_all[j]).astype(np.float16),
        })
    return in_maps, tuple(int(c) for c in C)


_CACHE = {}


def kernel(x, edge_index, Lsym, W_high, W_conv, b_conv, aL, aH):
    in_maps, C = _prepare_host(x, edge_index, Lsym, W_high, W_conv, b_conv,
                               aL, aH)
    nc = _CACHE.get(C)
    if nc is None:
        nc = _build_program(C)
        _CACHE[C] = nc
    res = run_bass_kernel_spmd(nc, in_maps, core_ids=list(range(NCORES)))
    # device output is partition-major [128, MB*D]; node b*128+p is at
    # [p, b*D:(b+1)*D]
    out = np.concatenate([
        np.asarray(res.results[j]["out"]).reshape(128, MB, D)
        .transpose(1, 0, 2).reshape(M, D)
        for j in range(NCORES)], axis=0)
    return out.astype(np.float32)


# revision 21
# speedup vs baseline: 1.0049x; 1.0049x over previous
"""FBGCN layer kernel for 8 Trainium2 NeuronCores.

out = aL * GCNConv(x, edge_index; W_conv, b_conv) + aH * (Lsym @ relu(x @ W_high.T))

Sharding: 1D row-partition of output nodes across 8 cores (1536 rows each).
The per-core cost is DMA-bandwidth bound (one serial 360 GB/s pipe in the
cost model), so the design minimizes total DMA bytes:

  - High-pass: stream this core's column slice of (256*aH*Lsym).T in
    fp8-e3m4 (18.9 MB, the irreducible floor) as the STATIONARY operand
    against moving fp16 Y = relu(x @ Wh.T) blocks. Output accumulates in
    12 per-block PSUM accumulators sharing banks (memset-seeded,
    start=False). Descale 1/256 on the PSUM->SBUF copy.
  - Low-pass GCN restructured as (S @ x) @ Wc.T with HOST-PACKED
    messages: the host already knows the full slot layout (it builds the
    seg matrix), so it packs msg[slot] = x[src(slot)] in fp8 as a dense
    [128, QBT*64] DRAM tensor. That replaces the old 36us
    256B-descriptor dma_gather (+ scratch round trip) with a ~5us
    full-bandwidth contiguous load. Per 128-target block: seg-matmul
    (fp8 x fp8) accumulates aggT[d,t] = sum_slots msg[s,d]*seg[s,t] in a
    [64,128] PSUM tile, then one [64]x[64,64] matmul applies Wc.T/16 and
    the host-built self-loop+bias term (sbias) is added.
  - Edges sorted by target, one slot per distinct (32-target group,
    source) pair; seg holds 16*aL*dinv[src]*dinv[tgt] in fp8 (G=32
    halves seg bytes vs G=64 at ~7% more slots). Pad slots are zero.
  - lsym loads are issued from the Activation engine's HWDGE queue;
    consts + xT (chunked so A0 starts early) from SP's queue.
No cross-core communication.
"""

import numpy as np

import concourse.bacc as bacc
import concourse.mybir as mybir
import concourse.tile as tile
from concourse.bass_utils import run_bass_kernel_spmd

N, E, D = 12288, 196608, 64
NCORES = 8
M = N // NCORES          # 1536 output rows per core
MB = M // 128            # 12 target blocks per core
KB = N // 128            # 96 contraction blocks
G = 32                   # target-group width
GPB = 128 // G           # groups per block
NG = M // G              # groups per core
SL = 256.0               # lsym fp8 scale (folded with aH on host)
SSEG = 16.0              # seg fp8 scale (descaled into wcs on host)
A0B = 8                  # kb blocks per A0 batch

F32 = mybir.dt.float32
F16 = mybir.dt.float16
E3 = mybir.dt.float8e3
AFT = mybir.ActivationFunctionType


def _build_program(chunk_counts, gcn_sched=None, do_a1=True, do_gcn=True,
                   ls_bufs=10, ls_pack=2, xt_chunks=6, msg_pieces=3,
                   seg_pieces=2, ls_eng="act", const_eng=None, out_eng=None,
                   fin_pieces=((0, 6), (6, 12)), taper=4, msg_f16=False):
    """chunk_counts: tuple of NG per-group chunk counts (same on all cores)."""
    C = list(chunk_counts)
    assert len(C) == NG and all(c >= 1 for c in C)
    coff = np.zeros(NG + 1, np.int64)
    coff[1:] = np.cumsum(C)
    QBT = int(coff[NG])              # total edge chunks per core
    MSGDT = F16 if msg_f16 else E3

    nc = bacc.Bacc("TRN2", target_bir_lowering=False, debug=False,
                   num_devices=NCORES)

    lsymT = nc.dram_tensor("lsymT", [N, M], E3, kind="ExternalInput")
    xT = nc.dram_tensor("xT", [D, N], F16, kind="ExternalInput")
    wt2 = nc.dram_tensor("wt2", [D, 2 * D], F16, kind="ExternalInput")
    msgT = nc.dram_tensor("msgT", [128, QBT * D], MSGDT,
                          kind="ExternalInput")
    segT = nc.dram_tensor("segT", [128, QBT * G], E3, kind="ExternalInput")
    sbiasT = nc.dram_tensor("sbiasT", [128, MB * D], F16,
                            kind="ExternalInput")
    # partition-major output (one contiguous 1536B line per partition -
    # full DMA bandwidth); host reassembles to [M, D]
    outp = nc.dram_tensor("out", [128, MB * D], F16, kind="ExternalOutput")

    ls_dma = {None: nc.sync, "act": nc.scalar}[ls_eng]
    const_dma = {None: nc.sync, "act": nc.scalar}[const_eng]
    out_dma = {None: nc.sync, "act": nc.scalar}[out_eng]

    # block boundaries (chunk index space), for piece-wise msg/seg loads
    bl_off = [int(coff[GPB * b]) for b in range(MB + 1)]

    def piece_bounds(npieces):
        """Split the MB blocks into npieces contiguous runs of blocks."""
        per = -(-MB // npieces)
        return [(bl_off[min(i * per, MB)], bl_off[min((i + 1) * per, MB)])
                for i in range(npieces)]

    with tile.TileContext(nc) as tc:
        with (
            tc.tile_pool(name="consts", bufs=1) as consts,
            tc.tile_pool(name="ls", bufs=ls_bufs) as ls_pool,
            tc.tile_pool(name="aggh", bufs=2) as agg_pool,
            tc.tile_pool(name="psb", bufs=1, space="PSUM") as ps_big,
            tc.tile_pool(name="psa", bufs=2, space="PSUM") as ps_a0,
            tc.tile_pool(name="psg", bufs=2, space="PSUM") as ps_agg,
        ):
            # ---- consts + xT (A0 is the critical-path prologue) ----
            wt2_sb = consts.tile([D, 2 * D], F16, tag="wt2")
            const_dma.dma_start(wt2_sb[:], wt2[:])
            # sbias early: it seeds the hh PSUM accumulator, which must be
            # ready before the first lsym-stream matmul
            sbias = consts.tile([128, MB * D], F16, tag="sbias")
            const_dma.dma_start(sbias[:], sbiasT[:])
            xt_sb = consts.tile([D, N], F16, tag="xt")
            assert KB % xt_chunks == 0
            XTC = N // xt_chunks
            for h in range(xt_chunks):
                nc.sync.dma_start(xt_sb[:, h * XTC:(h + 1) * XTC],
                                  xT[:, h * XTC:(h + 1) * XTC])
            msg_sb = consts.tile([128, QBT * D], MSGDT, tag="msg")
            for c0, c1 in piece_bounds(msg_pieces):
                const_dma.dma_start(msg_sb[:, c0 * D:c1 * D],
                                    msgT[:, c0 * D:c1 * D])
            seg_sb = consts.tile([128, QBT * G], E3, tag="seg")
            for c0, c1 in piece_bounds(seg_pieces):
                const_dma.dma_start(seg_sb[:, c0 * G:c1 * G],
                                    segT[:, c0 * G:c1 * G])
            y16 = consts.tile([128, KB * D], F16, tag="y16")
            ob_sb = consts.tile([128, MB * D], F16, tag="ob")

            # ---- phase A0: Y16 = relu(x@Wh.T)/SL fp16 ----
            # (the 1/SL lsym descale is folded in here so the finale can
            # add hhps + hl directly with no ACT descale hop)
            for bt in range(KB // A0B):
                ps = ps_a0.tile([128, A0B * D], F32, tag="psa")
                for i in range(A0B):
                    kb = bt * A0B + i
                    nc.tensor.matmul(
                        ps[:, i * D:(i + 1) * D],
                        lhsT=xt_sb[:, kb * 128:(kb + 1) * 128],
                        rhs=wt2_sb[:, 0:D],
                        start=True, stop=True,
                    )
                nc.scalar.activation(
                    y16[:, bt * A0B * D:(bt + 1) * A0B * D], ps[:], AFT.Relu,
                    scale=1.0 / SL)

            # ---- high-pass stream + GCN compute interleaved ----
            if gcn_sched is None:
                gcn_sched = [10 + 7 * i for i in range(MB)]
            assert len(gcn_sched) == MB
            sched = {}
            for b, k in enumerate(gcn_sched):
                sched.setdefault(k, []).append(b)

            # 12 per-block accumulators [128, 64] packed in 2 PSUM banks.
            # start=True would mark a whole 2KB zero region pending and
            # corrupt sibling accumulators, so seed + accumulate with
            # start=False (group check skipped). The seed is the host-built
            # self-loop + bias term sbias, and the GCN second-stage matmul
            # also accumulates here, so the finale is a plain fp16 copy.
            hhps = ps_big.tile([128, MB * D], F32, tag="hh")
            nc.vector.tensor_copy(hhps[:], sbias[:])
            hhv = hhps[:].rearrange("p (b f) -> p b f", b=MB)
            segv = seg_sb[:].rearrange("p (q t) -> p q t", t=G)
            msgv = msg_sb[:].rearrange("p (q f) -> p q f", f=D)

            def emit_gcn_block(b):
                # aggT[d, t] accumulator: all GPB groups share one PSUM
                # bank, so memset-seed + start=False (same trick as hhps).
                agg_ps = ps_agg.tile([64, 128], F32, tag="agg")
                nc.vector.memset(agg_ps[:], 0)
                for g in range(GPB):
                    gl = GPB * b + g
                    cg = C[gl]
                    for c in range(cg):
                        q = int(coff[gl]) + c
                        nc.tensor.matmul(
                            agg_ps[:, G * g:G * (g + 1)],
                            lhsT=msgv[:, q, :],
                            rhs=segv[:, q, :],
                            start=False, stop=(c == cg - 1),
                            skip_group_check=True)
                aggh = agg_pool.tile([64, 128], F16, tag="aggh")
                nc.scalar.activation(aggh[:], agg_ps[:], AFT.Copy)
                nc.tensor.matmul(hhv[:, b, :], lhsT=aggh[:],
                                 rhs=wt2_sb[:, D:2 * D],
                                 start=False, stop=False,
                                 skip_group_check=True)

            # lsym load schedule: packs of ls_pack, tapering to singles
            # before the piece-interleaved tail (last two chunks)
            assert taper >= 2
            pack_of = {}
            k = 0
            while k < KB - 2:
                size = ls_pack if k < KB - taper else 1
                pack_of[k] = size
                k += size
            ls_sb = None
            cur0 = 0
            for kb in range(KB - 2 if do_a1 else 0):
                if kb in pack_of:
                    size = pack_of[kb]
                    cur0 = kb
                    ls_sb = ls_pool.tile([128, ls_pack * M], E3, tag="ls")
                    ls_dma.dma_start(
                        ls_sb[:, 0:size * M]
                        .rearrange("p (t m) -> p t m", t=size),
                        lsymT[kb * 128:(kb + size) * 128, :]
                        .rearrange("(t p) m -> p t m", p=128),
                    )
                lsv = ls_sb[:].rearrange("p (t m) -> p t m", t=ls_pack)
                for b in range(MB):
                    nc.tensor.matmul(
                        hhv[:, b, :],
                        lhsT=lsv[:, kb - cur0, b * 128:(b + 1) * 128],
                        rhs=y16[:, kb * D:(kb + 1) * D],
                        start=False, stop=False,
                        skip_group_check=True,
                    )
                if do_gcn and kb in sched:
                    for b in sched[kb]:
                        emit_gcn_block(b)
            if do_gcn and not do_a1:
                for b in range(MB):
                    emit_gcn_block(b)

            # ---- tail: last two lsym chunks split at finale piece
            # boundaries and interleaved, PE matmuls emitted piece-first,
            # so each finale piece (fp16 copy + store) chains off the
            # earliest possible load ----
            if do_a1:
                ls94 = ls_pool.tile([128, ls_pack * M], E3, tag="ls")
                ls95 = ls_pool.tile([128, ls_pack * M], E3, tag="ls")
                tail = ((KB - 2, ls94), (KB - 1, ls95))
                for h0, h1 in fin_pieces:
                    for kb, lt in tail:
                        ls_dma.dma_start(
                            lt[:, h0 * 128:h1 * 128],
                            lsymT[kb * 128:(kb + 1) * 128, h0 * 128:h1 * 128]
                            .rearrange("(t p) m -> p (t m)", p=128),
                        )
            for h0, h1 in fin_pieces:
                if do_a1:
                    for kb, lt in tail:
                        for b in range(h0, h1):
                            nc.tensor.matmul(
                                hhv[:, b, :],
                                lhsT=lt[:, b * 128:(b + 1) * 128],
                                rhs=y16[:, kb * D:(kb + 1) * D],
                                start=False, stop=(kb == KB - 1),
                                skip_group_check=True,
                            )
                sl = slice(h0 * D, h1 * D)
                nc.scalar.activation(ob_sb[:, sl], hhps[:, sl], AFT.Copy)
                out_dma.dma_start(outp[:, sl], ob_sb[:, sl])

    nc.compile()
    return nc


def _prepare_host(x, edge_index, Lsym, W_high, W_conv, b_conv, aL, aH,
                  msg_f16=False):
    """Shard + preprocess inputs. Returns (in_maps, chunk_counts)."""
    import ml_dtypes
    E3NP = ml_dtypes.float8_e3m4
    MSGNP = np.float16 if msg_f16 else E3NP

    x = np.asarray(x, np.float32)
    edge_index = np.asarray(edge_index)
    Lsym = np.asarray(Lsym, np.float32)
    W_high = np.asarray(W_high, np.float32)
    W_conv = np.asarray(W_conv, np.float32)
    b_conv = np.asarray(b_conv, np.float32)
    aL = float(np.asarray(aL))
    aH = float(np.asarray(aH))

    src = edge_index[0].astype(np.int64)
    tgt = edge_index[1].astype(np.int64)

    # degrees with self loops (matches PyG GCNConv gcn_norm)
    deg = np.bincount(tgt, minlength=N).astype(np.float64) + 1.0
    dinv = 1.0 / np.sqrt(deg)
    w = (aL * dinv[src] * dinv[tgt]).astype(np.float32)
    wself = (aL * dinv * dinv).astype(np.float32)

    grp = tgt // G                    # global group id
    # dedupe (group, src): one msg slot serves every edge from the same
    # source into the group (weights land in different seg columns / sum)
    key = grp * N + src
    uk, inv = np.unique(key, return_inverse=True)
    ugrp = uk // N
    usrc = uk % N
    ucnt = np.bincount(ugrp, minlength=NCORES * NG).reshape(NCORES, NG)
    C = np.maximum(1, -(-ucnt.max(axis=0) // 128)).astype(np.int64)
    coff = np.zeros(NG + 1, np.int64)
    coff[1:] = np.cumsum(C)
    QBT = int(coff[NG])

    ustart = np.zeros(NCORES * NG, np.int64)
    ucnt_flat = np.bincount(ugrp, minlength=NCORES * NG)
    ustart[1:] = np.cumsum(ucnt_flat)[:-1]
    upos = np.arange(len(uk)) - ustart[ugrp]
    ucore = ugrp // NG
    ugl = ugrp % NG

    # host-packed messages: msg[slot] = x[src(slot)], pad slots zero
    x8 = x.astype(MSGNP)
    msg_all = np.zeros((NCORES, 128, QBT, D), MSGNP)
    chunk_u = coff[ugl] + upos // 128
    msg_all[ucore, upos % 128, chunk_u] = x8[usrc]
    msg_all = msg_all.reshape(NCORES, 128, QBT * D)

    # segment matrix, partition-major layout [128, QBT*G], value 16*w
    seg32 = np.zeros((NCORES, 128, QBT * G), np.float32)
    core_e = grp // NG
    pos_e = upos[inv]
    chunk_e = coff[grp % NG] + pos_e // 128
    np.add.at(seg32, (core_e, pos_e % 128, chunk_e * G + tgt % G),
              SSEG * w)
    segT_all = seg32.astype(E3NP)

    xT = np.ascontiguousarray(x.T).astype(np.float16)
    # wt2 = [W_high.T | W_conv.T/SSEG]; the GCN second-stage matmul applies
    # Wc.T with the 1/16 seg descale folded in
    wt2 = np.ascontiguousarray(np.concatenate(
        [W_high.T, W_conv.T / SSEG], axis=1)).astype(np.float16)
    # self-loop + bias term, host-side in fp32 (device cannot index its own
    # core's xw slice under SPMD): sbias[node] = wself*xw[node] + aL*b_conv
    xw_full = x @ W_conv.T.astype(np.float32)
    sb_full = wself[:, None] * xw_full + (aL * b_conv)[None, :]
    # [N, D] -> per core [128, MB*D] with node b*128+p at [p, b*D:(b+1)*D]
    sbias_all = sb_full.reshape(NCORES, MB, 128, D).transpose(0, 2, 1, 3) \
        .reshape(NCORES, 128, MB * D)

    in_maps = []
    for j in range(NCORES):
        lsymT_j = np.ascontiguousarray(
            (SL * aH * Lsym[j * M:(j + 1) * M, :]).T).astype(E3NP)
        in_maps.append({
            "lsymT": lsymT_j,
            "xT": xT,
            "wt2": wt2,
            "msgT": np.ascontiguousarray(msg_all[j]),
            "segT": np.ascontiguousarray(segT_all[j]),
            "sbiasT": np.ascontiguousarray(sbias_all[j]).astype(np.float16),
        })
    return in_maps, tuple(int(c) for c in C)


_CACHE = {}


def kernel(x, edge_index, Lsym, W_high, W_conv, b_conv, aL, aH):
    in_maps, C = _prepare_host(x, edge_index, Lsym, W_high, W_conv, b_conv,
                               aL, aH)
    nc = _CACHE.get(C)
    if nc is None:
        nc = _build_program(C)
        _CACHE[C] = nc
    res = run_bass_kernel_spmd(nc, in_maps, core_ids=list(range(NCORES)))
    # device output is partition-major [128, MB*D]; node b*128+p is at
    # [p, b*D:(b+1)*D]
    out = np.concatenate([
        np.asarray(res.results[j]["out"]).reshape(128, MB, D)
        .transpose(1, 0, 2).reshape(M, D)
        for j in range(NCORES)], axis=0)
    return out.astype(np.float32)


# revision 45
# speedup vs baseline: 1.0218x; 1.0169x over previous
"""FBGCN layer kernel for 8 Trainium2 NeuronCores.

out = aL * GCNConv(x, edge_index; W_conv, b_conv) + aH * (Lsym @ relu(x @ W_high.T))

Sharding: 1D row-partition of output nodes across 8 cores (1536 rows each).
The per-core cost is DMA-bandwidth bound (one serial 360 GB/s pipe in the
cost model), so the design minimizes total DMA bytes:

  - High-pass: stream this core's column slice of (256*aH*Lsym).T in
    fp8-e3m4 (18.9 MB, the irreducible floor) as the STATIONARY operand
    against moving fp16 Y = relu(x @ Wh.T) blocks. Output accumulates in
    12 per-block PSUM accumulators sharing banks (memset-seeded,
    start=False). Descale 1/256 on the PSUM->SBUF copy.
  - Low-pass GCN restructured as (S @ x) @ Wc.T with HOST-PACKED
    messages: the host already knows the full slot layout (it builds the
    seg matrix), so it packs msg[slot] = x[src(slot)] in fp8 as a dense
    [128, QBT*64] DRAM tensor. That replaces the old 36us
    256B-descriptor dma_gather (+ scratch round trip) with a ~5us
    full-bandwidth contiguous load. Per 128-target block: seg-matmul
    (fp8 x fp8) accumulates aggT[d,t] = sum_slots msg[s,d]*seg[s,t] in a
    [64,128] PSUM tile, then one [64]x[64,64] matmul applies Wc.T/16 and
    the host-built self-loop+bias term (sbias) is added.
  - Edges sorted by target, one slot per distinct (32-target group,
    source) pair; seg holds 16*aL*dinv[src]*dinv[tgt] in fp8 (G=32
    halves seg bytes vs G=64 at ~7% more slots). Pad slots are zero.
  - lsym loads are issued from the Activation engine's HWDGE queue;
    consts + xT (chunked so A0 starts early) from SP's queue.
No cross-core communication.
"""

import numpy as np

import concourse.bacc as bacc
import concourse.mybir as mybir
import concourse.tile as tile
from concourse.bass_utils import run_bass_kernel_spmd

N, E, D = 12288, 196608, 64
NCORES = 8
M = N // NCORES          # 1536 output rows per core
MB = M // 128            # 12 target blocks per core
KB = N // 128            # 96 contraction blocks
G = 32                   # target-group width
GPB = 128 // G           # groups per block
NG = M // G              # groups per core
SL = 256.0               # lsym fp8 scale (folded with aH on host)
SSEG = 16.0              # seg fp8 scale (descaled into wcs on host)
A0B = 8                  # kb blocks per A0 batch

F32 = mybir.dt.float32
F16 = mybir.dt.float16
E3 = mybir.dt.float8e3
AFT = mybir.ActivationFunctionType


def _gcn_layout(maxr):
    """Derive the shared (host+device) GCN slot/seg geometry from the
    per-group cross-core-max distinct-source counts.

    Slots are packed block-contiguously (no per-group 128-alignment): group
    gl's slots occupy block-local positions [O[gl], O[gl]+maxr[gl]); chunks
    are 128-slot rows of the block's run. A group spans chunks
    c0[gl]..c0[gl]+nch[gl]-1; its seg columns are a contiguous nch*G run at
    segoff[gl] (foreign slots in a shared chunk have zero seg weight).
    """
    O, c0, nch, chunks_b = [], [], [], []
    Qoff = [0]
    for b in range(MB):
        off = 0
        for g in range(GPB):
            gl = GPB * b + g
            O.append(off)
            c0.append(off // 128)
            nch.append((off + maxr[gl] - 1) // 128 - off // 128 + 1)
            off += maxr[gl]
        chunks_b.append(-(-off // 128))
        Qoff.append(Qoff[-1] + chunks_b[-1])
    segoff, scols = [], 0
    for gl in range(NG):
        segoff.append(scols)
        scols += nch[gl] * G
    segoff.append(scols)
    return O, chunks_b, Qoff, c0, nch, segoff, scols


def _build_program(chunk_counts, gcn_sched=None, do_a1=True, do_gcn=True,
                   ls_bufs=10, ls_pack=2, xt_chunks=6, msg_pieces=3,
                   seg_pieces=2, ls_eng="act", const_eng=None, out_eng=None,
                   fin_pieces=((0, 6), (6, 12)), taper=4, msg_f16=False):
    """chunk_counts: tuple of NG per-group max distinct-source counts
    (cross-core max, same layout on all cores)."""
    maxr = list(chunk_counts)
    assert len(maxr) == NG and all(r >= 1 for r in maxr)
    O, chunks_b, Qoff, c0, nch, segoff, SCOLS = _gcn_layout(maxr)
    QBT = Qoff[MB]                   # total msg chunks per core
    MSGDT = F16 if msg_f16 else E3

    nc = bacc.Bacc("TRN2", target_bir_lowering=False, debug=False,
                   num_devices=NCORES)

    lsymT = nc.dram_tensor("lsymT", [N, M], E3, kind="ExternalInput")
    xT = nc.dram_tensor("xT", [D, N], F16, kind="ExternalInput")
    wt2 = nc.dram_tensor("wt2", [D, 2 * D], F16, kind="ExternalInput")
    msgT = nc.dram_tensor("msgT", [128, QBT * D], MSGDT,
                          kind="ExternalInput")
    segT = nc.dram_tensor("segT", [128, SCOLS], E3, kind="ExternalInput")
    sbiasT = nc.dram_tensor("sbiasT", [128, MB * D], F16,
                            kind="ExternalInput")
    ident = nc.dram_tensor("ident", [128, 128], F16, kind="ExternalInput")
    # partition-major output (one contiguous 1536B line per partition -
    # full DMA bandwidth); host reassembles to [M, D]
    outp = nc.dram_tensor("out", [128, MB * D], F16, kind="ExternalOutput")

    ls_dma = {None: nc.sync, "act": nc.scalar}[ls_eng]
    const_dma = {None: nc.sync, "act": nc.scalar}[const_eng]
    out_dma = {None: nc.sync, "act": nc.scalar}[out_eng]

    def piece_bounds(npieces, off):
        """Split the MB blocks into npieces contiguous runs; return the
        [start, end) ranges of the per-block offset table `off`."""
        per = -(-MB // npieces)
        return [(off[min(i * per, MB)], off[min((i + 1) * per, MB)])
                for i in range(npieces)]

    seg_bl = [segoff[GPB * b] for b in range(MB)] + [SCOLS]

    with tile.TileContext(nc) as tc:
        with (
            tc.tile_pool(name="consts", bufs=1) as consts,
            tc.tile_pool(name="ls", bufs=ls_bufs) as ls_pool,
            tc.tile_pool(name="aggh", bufs=2) as agg_pool,
            tc.tile_pool(name="psb", bufs=1, space="PSUM") as ps_big,
            tc.tile_pool(name="psa", bufs=2, space="PSUM") as ps_a0,
            tc.tile_pool(name="psg", bufs=2, space="PSUM") as ps_agg,
        ):
            # ---- lsym pack 0 first on the SP queue: the stream is the
            # long pole, so its first bytes should hit the DMA pipe at the
            # earliest possible issue slot ----
            ls0_sb = ls_pool.tile([128, ls_pack * M], E3, tag="ls")
            nc.sync.dma_start(
                ls0_sb[:, 0:ls_pack * M]
                .rearrange("p (t m) -> p t m", t=ls_pack),
                lsymT[0:ls_pack * 128, :]
                .rearrange("(t p) m -> p t m", p=128),
            )
            # ---- consts + xT (A0 is the critical-path prologue) ----
            wt2_sb = consts.tile([D, 2 * D], F16, tag="wt2")
            const_dma.dma_start(wt2_sb[:], wt2[:])
            # sbias early: it seeds the hh PSUM accumulator, which must be
            # ready before the first lsym-stream matmul
            sbias = consts.tile([128, MB * D], F16, tag="sbias")
            const_dma.dma_start(sbias[:], sbiasT[:])
            id_sb = consts.tile([128, 128], F16, tag="id")
            const_dma.dma_start(id_sb[:], ident[:])
            xt_sb = consts.tile([D, N], F16, tag="xt")
            assert KB % xt_chunks == 0
            XTC = N // xt_chunks
            for h in range(xt_chunks):
                nc.sync.dma_start(xt_sb[:, h * XTC:(h + 1) * XTC],
                                  xT[:, h * XTC:(h + 1) * XTC])
            msg_sb = consts.tile([128, QBT * D], MSGDT, tag="msg")
            for p0, p1 in piece_bounds(msg_pieces, Qoff):
                const_dma.dma_start(msg_sb[:, p0 * D:p1 * D],
                                    msgT[:, p0 * D:p1 * D])
            seg_sb = consts.tile([128, SCOLS], E3, tag="seg")
            for p0, p1 in piece_bounds(seg_pieces, seg_bl):
                const_dma.dma_start(seg_sb[:, p0:p1], segT[:, p0:p1])
            y16 = consts.tile([128, KB * D], F16, tag="y16")
            ob_sb = consts.tile([128, MB * D], F16, tag="ob")

            # ---- seed the hh accumulator ON THE PE (identity matmul with
            # start=True, one per PSUM bank): PE in-order execution makes
            # the seed race-free vs all later accumulates. (A DVE-written
            # seed was observed to be flakily lost on HW.) ----
            hhps = ps_big.tile([128, MB * D], F32, tag="hh")
            BANKF = 512              # fp32 elems per 2KB PSUM bank
            for s0 in range(0, MB * D, BANKF):
                s1 = min(s0 + BANKF, MB * D)
                nc.tensor.matmul(hhps[:, s0:s1], lhsT=id_sb[:],
                                 rhs=sbias[:, s0:s1],
                                 start=True, stop=False,
                                 skip_group_check=True)

            # ---- phase A0: Y16 = relu(x@Wh.T)/SL fp16 ----
            # (the 1/SL lsym descale is folded in here so the finale can
            # add hhps + hl directly with no ACT descale hop)
            for bt in range(KB // A0B):
                ps = ps_a0.tile([128, A0B * D], F32, tag="psa")
                for i in range(A0B):
                    kb = bt * A0B + i
                    nc.tensor.matmul(
                        ps[:, i * D:(i + 1) * D],
                        lhsT=xt_sb[:, kb * 128:(kb + 1) * 128],
                        rhs=wt2_sb[:, 0:D],
                        start=True, stop=True,
                    )
                nc.scalar.activation(
                    y16[:, bt * A0B * D:(bt + 1) * A0B * D], ps[:], AFT.Relu,
                    scale=1.0 / SL)

            # ---- high-pass stream + GCN compute interleaved ----
            if gcn_sched is None:
                gcn_sched = [10 + 7 * i for i in range(MB)]
            assert len(gcn_sched) == MB
            sched = {}
            for b, k in enumerate(gcn_sched):
                sched.setdefault(k, []).append(b)

            # hhps holds 12 per-block accumulators [128, 64] packed in 2
            # PSUM banks, PE-seeded above with sbias; all stream/GCN
            # accumulates use start=False (a start=True would zero-pend a
            # whole 2KB bank and corrupt sibling accumulators mid-flight).
            hhv = hhps[:].rearrange("p (b f) -> p b f", b=MB)
            msgv = msg_sb[:].rearrange("p (q f) -> p q f", f=D)

            def emit_gcn_block(b):
                # aggT[d, t] accumulator: one PSUM bank; the block's very
                # first matmul start=True zero-pends the whole bank (all
                # GPB group regions), the rest accumulate start=False.
                agg_ps = ps_agg.tile([64, 128], F32, tag="agg")
                first = True
                for g in range(GPB):
                    gl = GPB * b + g
                    for j in range(nch[gl]):
                        q = Qoff[b] + c0[gl] + j
                        so = segoff[gl] + j * G
                        nc.tensor.matmul(
                            agg_ps[:, G * g:G * (g + 1)],
                            lhsT=msgv[:, q, :],
                            rhs=seg_sb[:, so:so + G],
                            start=first, stop=(j == nch[gl] - 1),
                            skip_group_check=True)
                        first = False
                aggh = agg_pool.tile([64, 128], F16, tag="aggh")
                nc.scalar.activation(aggh[:], agg_ps[:], AFT.Copy)
                nc.tensor.matmul(hhv[:, b, :], lhsT=aggh[:],
                                 rhs=wt2_sb[:, D:2 * D],
                                 start=False, stop=False,
                                 skip_group_check=True)

            # lsym load schedule: packs of ls_pack, tapering to singles
            # before the piece-interleaved tail (last two chunks)
            assert taper >= 2
            pack_of = {}
            k = 0
            while k < KB - 2:
                size = ls_pack if k < KB - taper else 1
                pack_of[k] = size
                k += size
            ls_sb = None
            cur0 = 0
            for kb in range(KB - 2 if do_a1 else 0):
                if kb in pack_of:
                    size = pack_of[kb]
                    cur0 = kb
                    if kb == 0 and size == ls_pack:
                        ls_sb = ls0_sb   # already issued up top
                    else:
                        ls_sb = ls_pool.tile([128, ls_pack * M], E3,
                                             tag="ls")
                        ls_dma.dma_start(
                            ls_sb[:, 0:size * M]
                            .rearrange("p (t m) -> p t m", t=size),
                            lsymT[kb * 128:(kb + size) * 128, :]
                            .rearrange("(t p) m -> p t m", p=128),
                        )
                lsv = ls_sb[:].rearrange("p (t m) -> p t m", t=ls_pack)
                for b in range(MB):
                    nc.tensor.matmul(
                        hhv[:, b, :],
                        lhsT=lsv[:, kb - cur0, b * 128:(b + 1) * 128],
                        rhs=y16[:, kb * D:(kb + 1) * D],
                        start=False, stop=False,
                        skip_group_check=True,
                    )
                if do_gcn and kb in sched:
                    for b in sched[kb]:
                        emit_gcn_block(b)
            if do_gcn and not do_a1:
                for b in range(MB):
                    emit_gcn_block(b)

            # ---- tail: last two lsym chunks split at finale piece
            # boundaries and interleaved, PE matmuls emitted piece-first,
            # so each finale piece (fp16 copy + store) chains off the
            # earliest possible load ----
            if do_a1:
                ls94 = ls_pool.tile([128, ls_pack * M], E3, tag="ls")
                ls95 = ls_pool.tile([128, ls_pack * M], E3, tag="ls")
                tail = ((KB - 2, ls94), (KB - 1, ls95))
                for h0, h1 in fin_pieces:
                    for kb, lt in tail:
                        ls_dma.dma_start(
                            lt[:, h0 * 128:h1 * 128],
                            lsymT[kb * 128:(kb + 1) * 128, h0 * 128:h1 * 128]
                            .rearrange("(t p) m -> p (t m)", p=128),
                        )
            for h0, h1 in fin_pieces:
                if do_a1:
                    for kb, lt in tail:
                        for b in range(h0, h1):
                            nc.tensor.matmul(
                                hhv[:, b, :],
                                lhsT=lt[:, b * 128:(b + 1) * 128],
                                rhs=y16[:, kb * D:(kb + 1) * D],
                                start=False, stop=(kb == KB - 1),
                                skip_group_check=True,
                            )
                sl = slice(h0 * D, h1 * D)
                nc.scalar.activation(ob_sb[:, sl], hhps[:, sl], AFT.Copy)
                out_dma.dma_start(outp[:, sl], ob_sb[:, sl])

    nc.compile()
    return nc


def _prepare_host(x, edge_index, Lsym, W_high, W_conv, b_conv, aL, aH,
                  msg_f16=False):
    """Shard + preprocess inputs. Returns (in_maps, chunk_counts)."""
    import ml_dtypes
    E3NP = ml_dtypes.float8_e3m4
    MSGNP = np.float16 if msg_f16 else E3NP

    x = np.asarray(x, np.float32)
    edge_index = np.asarray(edge_index)
    Lsym = np.asarray(Lsym, np.float32)
    W_high = np.asarray(W_high, np.float32)
    W_conv = np.asarray(W_conv, np.float32)
    b_conv = np.asarray(b_conv, np.float32)
    aL = float(np.asarray(aL))
    aH = float(np.asarray(aH))

    src = edge_index[0].astype(np.int64)
    tgt = edge_index[1].astype(np.int64)

    # degrees with self loops (matches PyG GCNConv gcn_norm)
    deg = np.bincount(tgt, minlength=N).astype(np.float64) + 1.0
    dinv = 1.0 / np.sqrt(deg)
    w = (aL * dinv[src] * dinv[tgt]).astype(np.float32)
    wself = (aL * dinv * dinv).astype(np.float32)

    grp = tgt // G                    # global group id
    # dedupe (group, src): one msg slot serves every edge from the same
    # source into the group (weights land in different seg columns / sum)
    key = grp * N + src
    uk, inv = np.unique(key, return_inverse=True)
    ugrp = uk // N
    usrc = uk % N
    ucnt = np.bincount(ugrp, minlength=NCORES * NG).reshape(NCORES, NG)
    maxr = tuple(int(r) for r in np.maximum(1, ucnt.max(axis=0)))
    O, chunks_b, Qoff, c0, nch, segoff, SCOLS = _gcn_layout(maxr)
    QBT = Qoff[MB]

    ustart = np.zeros(NCORES * NG, np.int64)
    ucnt_flat = np.bincount(ugrp, minlength=NCORES * NG)
    ustart[1:] = np.cumsum(ucnt_flat)[:-1]
    upos = np.arange(len(uk)) - ustart[ugrp]
    ucore = ugrp // NG
    ugl = ugrp % NG
    # block-contiguous slot position (block-local), then global chunk id
    On = np.asarray(O, np.int64)
    Qoffn = np.asarray(Qoff, np.int64)
    c0n = np.asarray(c0, np.int64)
    segoffn = np.asarray(segoff[:NG], np.int64)
    ubls = On[ugl] + upos             # block-local slot
    ublk = ugl // GPB
    uchunk = Qoffn[ublk] + ubls // 128

    # host-packed messages: msg[slot] = x[src(slot)], pad slots zero
    x8 = x.astype(MSGNP)
    msg_all = np.zeros((NCORES, 128, QBT, D), MSGNP)
    msg_all[ucore, ubls % 128, uchunk] = x8[usrc]
    msg_all = msg_all.reshape(NCORES, 128, QBT * D)

    # segment matrix [128, SCOLS]: group gl's columns at
    # segoff[gl] + (chunk-local j)*G + tgt%G, value 16*w
    seg32 = np.zeros((NCORES, 128, SCOLS), np.float32)
    core_e = grp // NG
    gl_e = grp % NG
    bls_e = ubls[inv]
    j_e = bls_e // 128 - c0n[gl_e]
    np.add.at(seg32, (core_e, bls_e % 128,
                      segoffn[gl_e] + j_e * G + tgt % G), SSEG * w)
    segT_all = seg32.astype(E3NP)

    xT = np.ascontiguousarray(x.T).astype(np.float16)
    # wt2 = [W_high.T | W_conv.T/SSEG]; the GCN second-stage matmul applies
    # Wc.T with the 1/16 seg descale folded in
    wt2 = np.ascontiguousarray(np.concatenate(
        [W_high.T, W_conv.T / SSEG], axis=1)).astype(np.float16)
    # self-loop + bias term, host-side in fp32 (device cannot index its own
    # core's xw slice under SPMD): sbias[node] = wself*xw[node] + aL*b_conv
    xw_full = x @ W_conv.T.astype(np.float32)
    sb_full = wself[:, None] * xw_full + (aL * b_conv)[None, :]
    # [N, D] -> per core [128, MB*D] with node b*128+p at [p, b*D:(b+1)*D]
    sbias_all = sb_full.reshape(NCORES, MB, 128, D).transpose(0, 2, 1, 3) \
        .reshape(NCORES, 128, MB * D)

    in_maps = []
    for j in range(NCORES):
        lsymT_j = np.ascontiguousarray(
            (SL * aH * Lsym[j * M:(j + 1) * M, :]).T).astype(E3NP)
        in_maps.append({
            "lsymT": lsymT_j,
            "xT": xT,
            "wt2": wt2,
            "msgT": np.ascontiguousarray(msg_all[j]),
            "segT": np.ascontiguousarray(segT_all[j]),
            "sbiasT": np.ascontiguousarray(sbias_all[j]).astype(np.float16),
            "ident": np.eye(128, dtype=np.float16),
        })
    return in_maps, maxr


_CACHE = {}


def kernel(x, edge_index, Lsym, W_high, W_conv, b_conv, aL, aH):
    in_maps, C = _prepare_host(x, edge_index, Lsym, W_high, W_conv, b_conv,
                               aL, aH)
    nc = _CACHE.get(C)
    if nc is None:
        nc = _build_program(C)
        _CACHE[C] = nc
    res = run_bass_kernel_spmd(nc, in_maps, core_ids=list(range(NCORES)))
    # device output is partition-major [128, MB*D]; node b*128+p is at
    # [p, b*D:(b+1)*D]
    out = np.concatenate([
        np.asarray(res.results[j]["out"]).reshape(128, MB, D)
        .transpose(1, 0, 2).reshape(M, D)
        for j in range(NCORES)], axis=0)
    return out.astype(np.float32)
